# revision 4
# baseline (speedup 1.0000x reference)
"""DWT-based Perona-Malik diffusion block on 8 Trainium2 NeuronCores. v2.

Math (see reference): one level of orthonormal Haar DWT, PM diffusion of the
detail subbands computed from batch 0 only and broadcast to every batch, IDWT,
then conv3x3 -> BN -> relu -> conv3x3 -> BN -> +feat.

Algebraic reductions used here:
  g   = 1 / (1 + LH^2 + HL^2)              (the sqrt cancels, K_PM = 1)
  With e = a - d and f = b - c of batch 0's 2x2 blocks (a,b,c,d = the four
  polyphase components):  LH^2 + HL^2 = (e^2 + f^2) / 2,
  u := (dLH + dHL)/2 = g*e/2,  v := (dLH - dHL)/2 = g*f/2.
  Per batch, with s1 = a + d, s2 = b + c:
    feat[2i,2j]   = s1/2 + u     feat[2i,2j+1] = s2/2 + v
    feat[2i+1,2j] = s2/2 - v     feat[2i+1,2j+1] = s1/2 - u
  BN folds into the conv weights/biases; the convs run as f32r matmuls over
  stacked (channel x row-parity) partitions.

Conv scheme (6 taps instead of 9): shift the OUTPUT row pairing by one row.
With input pairs (2j, 2j+1) on partitions (c, c+64), outputs are produced in
pairs (2j+1, 2j+2).  For each kx, output pair j needs only two matmuls:
  M1 (moving = input pair j):    out 2j+1 <- wc[ky0]@row2j + wc[ky1]@row2j+1
                                 out 2j+2 <- wc[ky0]@row2j+1
  M2 (moving = input pair j+1):  out 2j+1 <- wc[ky2]@row2j+2
                                 out 2j+2 <- wc[ky1]@row2j+2 + wc[ky2]@row2j+3
Conv2 consumes the shifted pairs and shifts again, restoring standard pairing
for the output, so no relayout is needed between the convs.

v2 performance structure (the v1 bottleneck was the SP DMA queue at 98%):
  - one merged DMA per chunk load / store (multi-level DRAM access patterns
    carry the (w-half, channel) / (row-parity, channel) partition composites)
  - DMA work spread over all three DGE queues (sync / scalar / gpsimd)
  - no cross-chunk fr/zt copy DMAs: conv1's last row-pair reads the next
    chunk's fr tile directly (per-row matmul splits), and conv1's activation
    double-writes z-pair 8k+8 into both zt(k) slot 8 and zt(k+1) slot 0.

Sharding: pure data parallelism, 2 batches per core; every core redundantly
computes u,v from batch 0 (x0 is shipped to all cores).
"""

import sys

for _p in ("/opt/pypackages", "/opt/trn_rl_repo"):
    if _p not in sys.path:
        sys.path.insert(0, _p)

import numpy as np

import concourse.bass as bass
import concourse.mybir as mybir
import concourse.tile as tile_mod
from concourse.bass_utils import run_bass_kernel_spmd
from concourse.tile import TileContext
from concourse.vector_clock import ScopedClock

F32 = mybir.dt.float32
F32R = mybir.dt.float32r
BF16 = mybir.dt.bfloat16
AF = mybir.ActivationFunctionType
ALU = mybir.AluOpType

N_CORES = 8
B, C, H, W = 16, 64, 256, 256
BPC = B // N_CORES  # batches per core
NPAIR = H // 2  # 128 row pairs per image
G = 8  # row pairs per chunk
NCHUNK = NPAIR // G  # 16
FS = G + 1  # zt slots per chunk (slot 0 = previous chunk's last z-pair)
EPS = 1e-5


def _patched_drain_and_barrier(self, tick_clock, wait_clock):
    # This walrus build rejects >1 sync-wait command per instruction; put the
    # tile-exit drain's waits on individual nops instead.
    nc = self.nc
    collector = nc.sync.nop(nofuse=True)
    wait_clock.add_sem_waits(
        collector.ins, ScopedClock({None: tick_clock.global_clock})
    )
    si = collector.ins.sync_info
    waits = list(si.on_wait) if si is not None else []
    if si is not None:
        si.on_wait = waits[:1]
    for w in waits[1:]:
        n = nc.sync.nop(nofuse=True)
        n.ins.sync_info = mybir.SyncInfo(on_wait=[w], on_update=[])
    nc.sync.drain()
    nc.all_engine_barrier()
    popped = nc._tile_sem_poison_stack.pop()
    assert popped is self._sem_poison
    nc.clear_and_free_semaphores(list(self.sems.allocated().values()))
    nc.all_engine_barrier()


tile_mod.TileContext._drain_and_barrier = _patched_drain_and_barrier

import concourse.bass_utils as _bu

if not getattr(_bu, "_ldw_opt_patched", False):
    _orig_run_command = _bu.run_command

    def _run_command_ldw_opt(argv, **kw):
        import os as _os

        if not _os.environ.get("NO_LDW_OPT"):
            argv = [
                "--enable-ldw-opt=true" if a == "--enable-ldw-opt=false" else a
                for a in argv
            ]
        return _orig_run_command(argv, **kw)

    _bu.run_command = _run_command_ldw_opt
    _bu._ldw_opt_patched = True


def split_multi_waits(nc):
    """Move extra sync-waits onto preceding single-wait nops (same engine)."""
    for fn in nc.m.functions:
        for blk in fn.blocks:
            new_insts = []
            for inst in blk.instructions:
                si = inst.sync_info
                waits = list(si.on_wait) if si is not None else []
                if len(waits) > 1:
                    for w in waits[:-1]:
                        n = mybir.InstNoOp(
                            name=nc.get_next_instruction_name(), ins=[], outs=[]
                        )
                        n.engine = inst.engine
                        n.bass_nofuse = True
                        n.sync_info = mybir.SyncInfo(on_wait=[w], on_update=[])
                        new_insts.append(n)
                    si.on_wait = waits[-1:]
                new_insts.append(inst)
            blk.instructions = new_insts


def _build_nc():
    nc = bass.Bass("TRN2", target_bir_lowering=False, debug=False,
                   num_devices=N_CORES)

    xs_d = nc.dram_tensor("xs", [BPC, C, H, W], F32, kind="ExternalInput").ap()
    x0_d = nc.dram_tensor("x0", [C, H, W], F32, kind="ExternalInput").ap()
    wts_d = nc.dram_tensor("wts", [128, 6, 128], F32R,
                           kind="ExternalInput").ap()
    wts2_d = nc.dram_tensor("wts2", [128, 6, 128], F32R,
                            kind="ExternalInput").ap()
    b1_d = nc.dram_tensor("bias1", [128, 1], F32, kind="ExternalInput").ap()
    b2_d = nc.dram_tensor("bias2", [128, 1], F32, kind="ExternalInput").ap()
    out_d = nc.dram_tensor("out", [BPC, C, H, W], F32,
                           kind="ExternalOutput").ap()

    with TileContext(nc) as tc, nc.allow_low_precision(
        reason="PM diffusion gain g and u,v detail averages tolerate bf16"
    ):
        with (
            tc.tile_pool(name="const", bufs=1) as cpool,
            tc.tile_pool(name="uv", bufs=1) as uvpool,
            tc.tile_pool(name="xin", bufs=2) as xpool,
            tc.tile_pool(name="x0in", bufs=2) as x0pool,
            tc.tile_pool(name="dtmp", bufs=2) as dpool,
            tc.tile_pool(name="qden", bufs=1) as qpool,
            tc.tile_pool(name="fasm", bufs=2) as fpool,
            tc.tile_pool(name="featR", bufs=4) as frpool,
            tc.tile_pool(name="zbuf", bufs=2) as zpool,
            tc.tile_pool(name="outb", bufs=2) as opool,
            tc.tile_pool(name="psum1", bufs=2, space="PSUM") as p1pool,
            tc.tile_pool(name="psum2", bufs=2, space="PSUM") as p2pool,
        ):
            wts_sb = cpool.tile([128, 6, 128], F32R)
            nc.sync.dma_start(out=wts_sb[:], in_=wts_d[:])
            wts2_sb = cpool.tile([128, 6, 128], F32R)
            nc.scalar.dma_start(out=wts2_sb[:], in_=wts2_d[:])
            b1_sb = cpool.tile([128, 1], F32)
            nc.sync.dma_start(out=b1_sb[:], in_=b1_d[:])
            b2_sb = cpool.tile([128, 1], F32)
            nc.sync.dma_start(out=b2_sb[:], in_=b2_d[:])
            # all-zero feat row-pair tile for the bottom image edge
            fr_zero = cpool.tile([128, 1, 258], F32R)
            nc.vector.memset(fr_zero[:].bitcast(F32), 0.0)

            # u,v in "assembly" layout: partition = ch + 64*(w half),
            # free = (row-pair index, DWT col within half)
            u_t = uvpool.tile([128, NPAIR, W // 4], BF16)
            v_t = uvpool.tile([128, NPAIR, W // 4], BF16)

            fr_tiles = {}
            z_tiles = {}
            x_tiles = {}
            x0_tiles = {}

            def load_chunk(bi, k, warm=False):
                rlo, rhi = 16 * k, 16 * k + 16
                # quad layout: partition = (w-half s, channel c),
                # free = (pair j, row-parity r, col-within-half w)
                xc = xpool.tile([128, G, 2, W // 2], F32)
                x_tiles[(bi, k)] = xc
                for s in range(2):
                    eng = nc.gpsimd if (warm and s == 0) else nc.sync
                    eng.dma_start(
                        out=xc[64 * s : 64 * (s + 1)],
                        in_=xs_d[
                            bi, :, rlo:rhi, 128 * s : 128 * (s + 1)
                        ].rearrange("c (j r) w -> c j r w", r=2),
                    )
                if bi == 0:
                    x0c = x0pool.tile([128, G, 2, W // 2], F32)
                    x0_tiles[k] = x0c
                    for s in range(2):
                        eng = (
                            nc.scalar if warm
                            else (nc.sync if s == 0 else nc.gpsimd)
                        )
                        eng.dma_start(
                            out=x0c[64 * s : 64 * (s + 1)],
                            in_=x0_d[
                                :, rlo:rhi, 128 * s : 128 * (s + 1)
                            ].rearrange("c (j r) w -> c j r w", r=2),
                        )

            def quads(t):
                return (t[:, :, 0, 0::2], t[:, :, 0, 1::2],
                        t[:, :, 1, 0::2], t[:, :, 1, 1::2])

            def prep_chunk(bi, k):
                uvsl = slice(G * k, G * (k + 1))
                if bi == 0:
                    x0c = x0_tiles.pop(k)
                    a, bq, cq, d = quads(x0c)
                    e_t = dpool.tile([128, G, W // 4], F32)
                    nc.vector.tensor_sub(out=e_t[:], in0=a, in1=d)
                    f_t = dpool.tile([128, G, W // 4], F32)
                    nc.vector.tensor_sub(out=f_t[:], in0=bq, in1=cq)
                    q1_t = qpool.tile([128, G, W // 4], F32)
                    nc.scalar.square(q1_t[:], e_t[:])
                    q2_t = qpool.tile([128, G, W // 4], F32)
                    nc.scalar.square(q2_t[:], f_t[:])
                    nc.vector.scalar_tensor_tensor(
                        out=q1_t[:], in0=q1_t[:],
                        scalar=1.0, in1=q2_t[:],
                        op0=ALU.mult, op1=ALU.add,
                    )
                    nc.vector.tensor_scalar(
                        out=q1_t[:], in0=q1_t[:],
                        scalar1=0.5, scalar2=1.0,
                        op0=ALU.mult, op1=ALU.add,
                    )
                    g_t = qpool.tile([128, G, W // 4], F32, name="g_t")
                    nc.vector.reciprocal(out=g_t[:], in_=q1_t[:])
                    nc.vector.scalar_tensor_tensor(
                        out=u_t[:, uvsl, :], in0=g_t[:], scalar=0.5,
                        in1=e_t[:], op0=ALU.mult, op1=ALU.mult,
                    )
                    nc.vector.scalar_tensor_tensor(
                        out=v_t[:, uvsl, :], in0=g_t[:], scalar=0.5,
                        in1=f_t[:], op0=ALU.mult, op1=ALU.mult,
                    )

                xc = x_tiles.pop((bi, k))
                a, bq, cq, d = quads(xc)
                s1_t = dpool.tile([128, G, W // 4], F32)
                nc.vector.tensor_add(out=s1_t[:], in0=a, in1=d)
                s2_t = dpool.tile([128, G, W // 4], F32)
                nc.vector.tensor_add(out=s2_t[:], in0=bq, in1=cq)

                fa = fpool.tile([128, G, 2, W // 2], F32)
                for (src, uv, r, par, op1) in (
                    (s1_t, u_t, 0, 0, ALU.add),
                    (s2_t, v_t, 0, 1, ALU.add),
                    (s2_t, v_t, 1, 0, ALU.subtract),
                    (s1_t, u_t, 1, 1, ALU.subtract),
                ):
                    nc.vector.scalar_tensor_tensor(
                        out=fa[:, :, r, par::2], in0=src[:],
                        scalar=0.5, in1=uv[:, uvsl, :],
                        op0=ALU.mult, op1=op1,
                    )

                # fr slot s (s=0..7) = row-pair 8k+s; conv1 reads the next
                # chunk's slot 0 directly for its last output pair.
                fr = frpool.tile([128, G, 258], F32R)
                fr_tiles[(bi, k)] = fr
                nc.vector.memset(fr[:, :, 0:1].bitcast(F32), 0.0)
                nc.vector.memset(fr[:, :, 257:258].bitcast(F32), 0.0)
                # relayout (w-half, ch) -> (row-parity, ch): one DMA per
                # (s, rp) quadrant, spread across the three DGE queues
                for s in range(2):
                    for rp in range(2):
                        eng = (nc.gpsimd, nc.gpsimd, nc.scalar, nc.scalar)[
                            2 * s + rp
                        ]
                        eng.dma_start(
                            out=fr[
                                64 * rp : 64 * (rp + 1), 0:G,
                                1 + 128 * s : 1 + 128 * (s + 1),
                            ].bitcast(F32),
                            in_=fa[64 * s : 64 * (s + 1), :, rp, :],
                        )

            def conv1_chunk(bi, k):
                fr = fr_tiles[(bi, k)]
                frn = fr_tiles[(bi, k + 1)] if k + 1 < NCHUNK else fr_zero
                # zt slot s = z-pair 8k+s (z rows 16k+2s-1, 16k+2s); slot 0
                # is written by the PREVIOUS chunk's conv1 (activation
                # double-write), or by the edge matmul for k == 0.
                if k == 0:
                    zt = zpool.tile([128, FS, 258], F32R)
                    z_tiles[(bi, 0)] = zt
                else:
                    zt = z_tiles[(bi, k)]
                if k + 1 < NCHUNK:
                    ztn = zpool.tile([128, FS, 258], F32R)
                    z_tiles[(bi, k + 1)] = ztn
                    # halo columns of the next tile's slot 0 (its data
                    # columns are double-written by this chunk's act)
                    nc.vector.memset(ztn[:, 0:1, 0:1].bitcast(F32), 0.0)
                    nc.vector.memset(ztn[:, 0:1, 257:258].bitcast(F32), 0.0)
                lo = 0 if k == 0 else 1
                nc.vector.memset(zt[:, lo:FS, 0:1].bitcast(F32), 0.0)
                nc.vector.memset(zt[:, lo:FS, 257:258].bitcast(F32), 0.0)
                ps = [p1pool.tile([128, 4, 256], F32, name="cps1")
                      for _h in range(2)]
                for t in range(3):
                    for m, soff in ((0, 0), (3, 1)):  # M1, M2
                        w_ap = wts_sb[:, m + t, :]
                        for h in range(2):
                            for bb in range(2):
                                q = 4 * h + 2 * bb
                                if q < 6 or m == 0:
                                    nc.tensor.matmul(
                                        ps[h][:, 2 * bb : 2 * bb + 2, :],
                                        w_ap,
                                        fr[:, q + soff : q + soff + 2,
                                           t : t + 256],
                                        start=(t == 0 and m == 0),
                                        stop=(t == 2 and m == 3),
                                    )
                                else:
                                    # q == 6, M2 straddles the chunk edge:
                                    # psum row 6 <- fr slot 7, psum row 7 <-
                                    # the NEXT chunk's fr slot 0
                                    for r in range(2):
                                        src = (
                                            fr[:, 7:8, t : t + 256]
                                            if r == 0
                                            else frn[:, 0:1, t : t + 256]
                                        )
                                        nc.tensor.matmul(
                                            ps[1][:, 2 + r : 3 + r, :],
                                            w_ap,
                                            src,
                                            start=False,
                                            stop=(t == 2 and r == 1),
                                        )
                for h in range(2):
                    nc.scalar.activation(
                        zt[:, 1 + 4 * h : 5 + 4 * h, 1:257],
                        ps[h][:],
                        AF.Relu, bias=b1_sb[:, 0:1], scale=1.0,
                    )
                if k + 1 < NCHUNK:
                    # double-write z-pair 8k+8 into the next tile's slot 0
                    nc.scalar.activation(
                        z_tiles[(bi, k + 1)][:, 0:1, 1:257],
                        ps[1][:, 3:4, :],
                        AF.Relu, bias=b1_sb[:, 0:1], scale=1.0,
                    )
                if k == 0:
                    # z-pair 0: row -1 (zeroed below) + row 0 via M2 only.
                    ps_e = p2pool.tile([128, 1, 256], F32, name="cps2")
                    for t in range(3):
                        nc.tensor.matmul(
                            ps_e[:],
                            wts_sb[:, 3 + t, :],
                            fr[:, 0:1, t : t + 256],
                            start=(t == 0),
                            stop=(t == 2),
                        )
                    nc.scalar.activation(
                        zt[:, 0:1, 1:257], ps_e[:],
                        AF.Relu, bias=b1_sb[:, 0:1], scale=1.0,
                    )
                    nc.vector.memset(zt[0:64, 0:1, :].bitcast(F32), 0.0)
                if k == NCHUNK - 1:
                    # bottom half of the last z-pair is image row 256: zero it
                    nc.vector.memset(
                        zt[64:128, G : G + 1, :].bitcast(F32), 0.0
                    )

            def conv2_chunk(bi, n):
                zt = z_tiles[(bi, n)]
                fr = fr_tiles[(bi, n)]
                ot = opool.tile([128, G, 256], F32)
                ps = [p2pool.tile([128, 4, 256], F32, name="cps2")
                      for _h in range(2)]
                for t in range(3):
                    for m, soff in ((6, 0), (9, 1)):  # M1', M2'
                        w_ap = wts2_sb[:, m - 6 + t, :]
                        for h in range(2):
                            for bb in range(2):
                                q = 4 * h + 2 * bb
                                nc.tensor.matmul(
                                    ps[h][:, 2 * bb : 2 * bb + 2, :],
                                    w_ap,
                                    zt[:, q + soff : q + soff + 2,
                                       t : t + 256],
                                    start=(t == 0 and m == 6),
                                    stop=(t == 2 and m == 9),
                                )
                for h in range(2):
                    nc.vector.scalar_tensor_tensor(
                        out=ot[:, 4 * h : 4 * h + 4, :], in0=ps[h][:],
                        scalar=b2_sb[:, 0:1],
                        in1=fr[:, 4 * h : 4 * h + 4, 1:257],
                        op0=ALU.add, op1=ALU.add,
                    )
                orows = slice(2 * G * n, 2 * G * (n + 1))
                for rp in range(2):
                    nc.gpsimd.dma_start(
                        out=out_d[bi, :, orows, :].rearrange(
                            "c (j r) w -> c j r w", r=2
                        )[:, :, rp, :],
                        in_=ot[64 * rp : 64 * (rp + 1)],
                    )
                del z_tiles[(bi, n)]
                del fr_tiles[(bi, n)]

            jobs = [(bi, k) for bi in range(BPC) for k in range(NCHUNK)]
            for j in range(-3, len(jobs) + 1):
                if 0 <= j + 2 < len(jobs):
                    prep_chunk(*jobs[j + 2])
                if 0 <= j + 3 < len(jobs):
                    load_chunk(*jobs[j + 3], warm=(j + 3 < 3))
                if 0 <= j - 1 < len(jobs):
                    conv2_chunk(*jobs[j - 1])
                if 0 <= j < len(jobs):
                    conv1_chunk(*jobs[j])

    split_multi_waits(nc)
    return nc


_NC_CACHE = {}


def _get_nc():
    if "nc" not in _NC_CACHE:
        _NC_CACHE["nc"] = _build_nc()
    return _NC_CACHE["nc"]


def _host_prep(w1, b1, g1, be1, m1, v1, w2, b2, g2, be2, m2, v2):
    inv1 = (g1 / np.sqrt(v1 + EPS)).astype(np.float64)
    inv2 = (g2 / np.sqrt(v2 + EPS)).astype(np.float64)
    wc1 = w1.astype(np.float64) * inv1[:, None, None, None]
    wc2 = w2.astype(np.float64) * inv2[:, None, None, None]
    b1p = (be1.astype(np.float64) + (b1.astype(np.float64) - m1) * inv1)
    b2p = (be2.astype(np.float64) + (b2.astype(np.float64) - m2) * inv2)

    wts = np.zeros((128, 12, 128), np.float32)
    for conv, wc in ((0, wc1), (1, wc2)):
        base = 6 * conv
        for kx in range(3):
            w0 = wc[:, :, 0, kx].T.astype(np.float32)
            w1k = wc[:, :, 1, kx].T.astype(np.float32)
            w2k = wc[:, :, 2, kx].T.astype(np.float32)
            # M1: moving = input pair j (rows 2j, 2j+1)
            wts[0:64, base + kx, 0:64] = w0
            wts[64:128, base + kx, 0:64] = w1k
            wts[64:128, base + kx, 64:128] = w0
            # M2: moving = input pair j+1 (rows 2j+2, 2j+3)
            wts[0:64, base + 3 + kx, 0:64] = w2k
            wts[0:64, base + 3 + kx, 64:128] = w1k
            wts[64:128, base + 3 + kx, 64:128] = w2k
    wts1 = np.ascontiguousarray(wts[:, 0:6, :])
    wts2 = np.ascontiguousarray(wts[:, 6:12, :])
    bias1 = np.tile(b1p.astype(np.float32), 2).reshape(128, 1)
    bias2 = np.tile(b2p.astype(np.float32), 2).reshape(128, 1)
    return wts1, wts2, bias1, bias2


def _build_in_maps(x, wts1, wts2, bias1, bias2):
    x0 = np.ascontiguousarray(x[0])
    in_maps = []
    for c in range(N_CORES):
        in_maps.append(
            {
                "xs": np.ascontiguousarray(x[BPC * c : BPC * (c + 1)]),
                "x0": x0,
                "wts": wts1,
                "wts2": wts2,
                "bias1": bias1,
                "bias2": bias2,
            }
        )
    return in_maps


def kernel(x, w1, b1, g1, be1, m1, v1, w2, b2, g2, be2, m2, v2, **_kw):
    x = np.ascontiguousarray(np.asarray(x, dtype=np.float32))
    wts1, wts2, bias1, bias2 = _host_prep(
        np.asarray(w1), np.asarray(b1), np.asarray(g1), np.asarray(be1),
        np.asarray(m1), np.asarray(v1), np.asarray(w2), np.asarray(b2),
        np.asarray(g2), np.asarray(be2), np.asarray(m2), np.asarray(v2),
    )
    in_maps = _build_in_maps(x, wts1, wts2, bias1, bias2)
    nc = _get_nc()
    try:
        res = run_bass_kernel_spmd(nc, in_maps, list(range(N_CORES)))
    except Exception:
        import time as _time

        _time.sleep(5)
        res = run_bass_kernel_spmd(nc, in_maps, list(range(N_CORES)))
    out = np.concatenate([r["out"] for r in res.results], axis=0)
    return out
